# revision 49
# baseline (speedup 1.0000x reference)
"""MoE routed-classification kernel for Trainium2 (8 NeuronCores, SPMD).

Problem: nn_DINOMIMICClassification — E=16 experts, each a 3-layer MLP
(D=1536 -> H=768 -> H=768 -> T=2, relu after layers 1/2); every sample of
the B=512 batch goes through the expert selected by head_idx[b].

Strategy (expert-parallel, host routing, mixed fp8/bf16 weights):
  - Each of the 8 cores owns 2 experts and receives only the samples routed
    to them (host groups samples by expert, pads each group to CAP=48
    columns; per-expert counts for the fixed input seed max out at 47).
  - The kernel is HBM-stream-bound, so weight BYTES are the roofline:
      * W1 ships as fp8 e3m4 (4 mantissa bits, x64 pre-scale to clear the
        format's subnormal range). The PE accepts fp8-stationary x
        bf16-moving matmuls, so x/h stay bf16.
      * W2 is importance-aware mixed precision: with only T=2 outputs, a
        W2 column's quantization error reaches the output weighted by
        |W3[col]|. The host permutes W2 columns by ascending |W3| row-norm
        (free: layer 3 runs on the host with the permuted W3); the 640
        least-important columns ship as e3m4, the top 128 as bf16.
    Total 4.0 MB/core (vs 7.4 all-bf16). Measured end-to-end rel err
    1.73e-2 against the 2e-2 gate (all-bf16 scores 4.0e-3, all-e3m4 1.95e-2
    without the importance split, 2.07e-2 with an unsorted split).
  - ALL weight chunks ride ONE queue (sync/SP HWDGE), emitted in exactly
    the PE's consumption order; the PE runs one chunk behind the stream.
    Chunk rows are per-partition contiguous (0.8-4.6 KB) for DMA line rate.
  - The PE is ldweights-bound at roughly the stream rate, so W1 is split in
    2 chunks/expert (early PE start) and layer 2 runs expert-serial with
    the bf16 tile second-to-last: after the final weight byte only 6 fp8
    matmuls + a [128,1,48] relu + a 12KB output DMA remain.
  - Layer 3 (768 -> 2) runs on the HOST: the kernel DMAs relu(layer2) back
    (147 KB bf16 total) and the host does the [n,768]@[768,2] einsum in
    fp32. Expert 0's output ships mid-stream on the ACT queue; expert 1's
    bulk ships on ACT while its tail computes, and the final 12 KB rides
    the by-then-idle SP queue so the two issue latencies overlap.
  - Each layer-2 chunk accumulates into its OWN 1-bank PSUM tile (a shared
    tile would serialize matmul->relu->matmul via tile-granular WAR); the
    epilogue is one DVE tensor_scalar max-with-0 (relu + f32->bf16 cast).
  - b1/b2 are zeros for this problem's inputs (asserted); b3 is added on
    the host.
"""

import os

import numpy as np

# Model dims (hardcoded; the grading harness calls kernel() standalone).
E, B, D, H, T = 16, 512, 1536, 768, 2
NCORES = 8
EPC = E // NCORES  # experts per core = 2
CAP = 48  # per-expert routed-sample capacity (actual max is 47)
W1SCALE = 64.0  # pre-scale so fp8(e3m4) W1 uses the format's normal range
KD = D // 128  # 12 contraction tiles for layer 1
KH = H // 128  # 6 contraction tiles for layers 2/3
# W2 mixed precision, importance-aware: the head has only T=2 outputs, so a
# W2 column's quantization error reaches the output weighted by |W3[col]|.
# Host permutes columns by ascending |W3| row-norm; the 5 least-important
# mh-tiles (640 cols) ship as fp8 e3m4, the top tile (128 cols) as bf16.
# Measured end-to-end rel err 1.73e-2 (gate 2e-2); unsorted split would be
# 2.07e-2. The bf16 tile streams LAST per expert, so after the final weight
# byte only 6 matmuls + a [128,1,48] relu + a 12KB DMA remain.
MH2Q = 5  # e3m4 mh-tiles per expert

_CACHE = {}


def _build_program():
    """Build the (single, SPMD) Bass program run on every core."""
    from contextlib import ExitStack

    import concourse.mybir as mybir
    import concourse.tile as tile
    from concourse import bacc

    f32 = mybir.dt.float32
    bf16 = mybir.dt.bfloat16
    f8e3 = mybir.dt.float8e3
    # Bacc (not raw Bass): its compile() legalization splits multi-sem waits
    # into EventSemaphore sequencer ops — TPB instructions have a single
    # hardware wait slot and walrus rejects >1 ("Too many sync wait commands").
    nc = bacc.Bacc("TRN2")

    # xg[p, e, kd, c]: bf16 routed samples, transposed per expert
    xg = nc.dram_tensor("xg", [128, EPC, KD, CAP], bf16, kind="ExternalInput")
    # w1g[e, p, (mh, kd*128+h)] = e3m4 of W1SCALE*W1[ge, kd*128+p, mh*128+h].
    # fp8 e3m4 (4 mantissa bits) halves W1's HBM traffic vs bf16; the PE
    # accepts mixed fp8-weights x bf16-moving matmuls. Two chunks per
    # expert: the PE is ldweights-bound at roughly the DMA stream rate, so
    # it must start on e0's first half as early as possible or it finishes
    # ~1.3us after the last weight byte.
    w1g = nc.dram_tensor("w1g", [EPC, 128, KH * KD * 128], f8e3, kind="ExternalInput")
    # w2g8/w2g16[e, p, (mh, kh*128+h)] = W2SCALE*W2[ge, kh*128+p, perm[mh*128+h]]:
    # flat per-expert rows; mh counts permuted-column tiles (0..4 fp8, 5 bf16).
    w2g8 = nc.dram_tensor("w2g8", [EPC, 128, MH2Q * KH * 128], f8e3, kind="ExternalInput")
    w2g16 = nc.dram_tensor("w2g16", [EPC, 128, (KH - MH2Q) * KH * 128], bf16, kind="ExternalInput")
    # hg[p, e, kh, c] = relu(layer2) activations, feature kh*128+p
    hg = nc.dram_tensor("hg", [128, EPC, KH, CAP], bf16, kind="ExternalOutput")

    with tile.TileContext(nc) as tc, ExitStack() as ctx:
        const_pool = ctx.enter_context(tc.tile_pool(name="const", bufs=1))
        # bufs >= number of live tiles per tag: any reuse would add a WAR
        # wait that stalls the in-order weight queue mid-stream.
        w1_pool = ctx.enter_context(tc.tile_pool(name="w1", bufs=3))
        w2_pool = ctx.enter_context(tc.tile_pool(name="w2", bufs=5))
        h_pool = ctx.enter_context(tc.tile_pool(name="h", bufs=EPC))

        # 2 layer-1 tiles + 5 layer-2 chunk tiles: every PSUM allocation
        # lives in its own bank (7 of 8); no buffer reuse -> no WAR stalls.
        psL_pool = ctx.enter_context(tc.tile_pool(name="psL", bufs=7, space="PSUM"))

        # x ships per expert: e0's slice leads the queue (the PE's first
        # matmul waits on it), e1's rides between the W1 chunks where the
        # PE is busy anyway. Total DMA time is queue-order-invariant.
        xsb = const_pool.tile([128, EPC, KD, CAP], bf16)
        nc.sync.dma_start(out=xsb[:, 0], in_=xg[:, 0])

        # Weight stream: ONE queue (sync), emitted in consumption order.
        # Layer-2 chunks run expert-serial (all of e0, then e1) so e0's
        # output DMA overlaps e1's weight stream and only e1's small bf16
        # tail remains after the last byte. Every chunk has its own buffer;
        # all DMAs are issued up front (no reuse waits).
        # W1 chunking is asymmetric: e0 gets a small 1-mh-tile head chunk so
        # the ldweights-bound PE starts as early as possible; everything
        # after rides maximal-size chunks (bigger descriptors = higher DMA
        # rate; mid-stream the PE runs behind the DMA anyway).
        W1CH = ((1, 5), (6,))
        w1sb = []
        for e in range(EPC):
            chunks = []
            off = 0
            for mhc in W1CH[e]:
                t = w1_pool.tile([128, mhc, KD * 128], f8e3, tag="w1", name=f"w1_{e}_{off}")
                lo = off * KD * 128
                nc.sync.dma_start(out=t, in_=w1g[e][:, lo : lo + mhc * KD * 128])
                chunks.append((t, off, mhc))
                off += mhc
            if e == 0:
                # e1's x slice rides after e0's W1 (PE busy with e0 by then)
                nc.sync.dma_start(out=xsb[:, 1], in_=xg[:, 1])
            w1sb.append(chunks)
        # Per-expert W2 chunk order: [f8 mh0-3][bf16 mh5][f8 mh4]. The LAST
        # chunk is fp8 (ldweights at 2 cols/cycle, ~half the bf16 cost), so
        # after the final byte only 6 cheap matmuls + a small relu remain.
        # h2 rows are assigned in COMPUTE order (f8 mh0-3 -> rows 0-3, bf16
        # -> row 4, f8 mh4 -> row 5), so the "everything but the last
        # chunk" output slice hg[:, e, :5, :] is contiguous. The host's
        # permuted-W3 row order matches this mapping.
        w2sb = []  # [e] -> list of (tile, h2-row-offset, mh-count)
        for e in range(EPC):
            chunks = []
            if e < EPC - 1:
                # not the global tail: one maximal fp8 chunk + the bf16 tile
                t8 = w2_pool.tile([128, MH2Q, KH * 128], f8e3, tag="w2", name=f"w2q_{e}")
                nc.sync.dma_start(out=t8, in_=w2g8[e])
                chunks.append((t8, 0, MH2Q))
                t16 = w2_pool.tile([128, KH - MH2Q, KH * 128], bf16, tag="w2", name=f"w2b_{e}")
                nc.sync.dma_start(out=t16, in_=w2g16[e])
                chunks.append((t16, MH2Q, KH - MH2Q))
            else:
                # global tail expert: bf16 second-to-last, small fp8 last
                t8a = w2_pool.tile([128, MH2Q - 1, KH * 128], f8e3, tag="w2", name=f"w2qa_{e}")
                nc.sync.dma_start(out=t8a, in_=w2g8[e][:, : (MH2Q - 1) * KH * 128])
                chunks.append((t8a, 0, MH2Q - 1))
                t16 = w2_pool.tile([128, KH - MH2Q, KH * 128], bf16, tag="w2", name=f"w2b_{e}")
                nc.sync.dma_start(out=t16, in_=w2g16[e])
                chunks.append((t16, MH2Q - 1, KH - MH2Q))
                t8b = w2_pool.tile([128, 1, KH * 128], f8e3, tag="w2", name=f"w2qb_{e}")
                nc.sync.dma_start(out=t8b, in_=w2g8[e][:, (MH2Q - 1) * KH * 128 :])
                chunks.append((t8b, KH - 1, 1))
            w2sb.append(chunks)

        # ---- layer 1 (both experts), relu epilogue. Consumption matches
        # the DMA emission order so the PE runs one chunk behind the stream.
        h1 = [h_pool.tile([128, KH, CAP], bf16, tag="h", name=f"h1_{e}") for e in range(EPC)]
        PS1 = [psL_pool.tile([128, KH, CAP], f32, tag="psL", name=f"ps1_{e}") for e in range(EPC)]
        for e in range(EPC):
            for w, off, mhc in w1sb[e]:
                for j in range(mhc):
                    for k in range(KD):
                        nc.tensor.matmul(
                            PS1[e][:, off + j, :],
                            w[:, j, k * 128 : (k + 1) * 128],
                            xsb[:, e, k, :],
                            start=(k == 0),
                            stop=(k == KD - 1),
                        )
            # relu with implicit f32->bf16 cast
            nc.vector.tensor_scalar_max(h1[e], PS1[e], 0.0)

        # ---- layer 2, expert-serial, per-chunk relu epilogue + output DMA.
        # Each chunk gets its OWN psum tile: a shared tile would give the
        # next chunk's matmuls a tile-granular WAR wait on this chunk's
        # relu, serializing matmul->relu->matmul at the stream tail.
        h2 = const_pool.tile([128, EPC, KH, CAP], bf16, tag="h2")
        for e in range(EPC):
            for ci, (w, off, mhc) in enumerate(w2sb[e]):
                ps = psL_pool.tile([128, mhc, CAP], f32, tag="psL", name=f"ps2_{e}_{off}")
                for j in range(mhc):
                    for k in range(KH):
                        nc.tensor.matmul(
                            ps[:, j, :],
                            w[:, j, k * 128 : (k + 1) * 128],
                            h1[e][:, k, :],
                            start=(k == 0),
                            stop=(k == KH - 1),
                        )
                # (Running the last chunk's relu on GpSimd to dodge the DVE
                # queue was tried: walrus rejects it - no GpSimd PSUM path.)
                nc.vector.tensor_scalar_max(h2[:, e, off : off + mhc, :], ps, 0.0)
            if e < EPC - 1:
                # whole expert ships right after its last relu, on the ACT
                # queue (the SP queue is still carrying e1's weights)
                nc.scalar.dma_start(out=hg[:, e, :, :], in_=h2[:, e, :, :])
            else:
                # last expert: bulk (5 mh-tiles) ships on ACT while the bf16
                # tail chunk computes; the final 12KB rides the (by now
                # idle) SP queue so the two DMAs' issue latencies overlap.
                # (Emitting these OUTSIDE the TileContext to overlap the
                # end-of-NEFF teardown was tried and crashes walrus codegen.)
                nc.scalar.dma_start(out=hg[:, e, :MH2Q, :], in_=h2[:, e, :MH2Q, :])
                nc.sync.dma_start(out=hg[:, e, MH2Q:, :], in_=h2[:, e, MH2Q:, :])

    nc.finalize()
    return nc


def _get_program():
    if "nc" not in _CACHE:
        _CACHE["nc"] = _build_program()
    return _CACHE["nc"]


def kernel(x, head_idx, W1, b1, W2, b2, W3, b3):
    # Make sure the axon jax platform is reachable (the Bass program executes
    # via PJRT on the 8 tunneled NeuronCores).
    if os.environ.get("JAX_PLATFORMS") not in (None, ""):
        if "axon" not in os.environ["JAX_PLATFORMS"]:
            os.environ["JAX_PLATFORMS"] = ""

    import ml_dtypes

    from concourse.bass_utils import run_bass_kernel_spmd

    bf16 = ml_dtypes.bfloat16
    x = np.ascontiguousarray(np.asarray(x, dtype=np.float32))
    head_idx = np.asarray(head_idx, dtype=np.int32)
    W1 = np.asarray(W1, dtype=np.float32)
    b1 = np.asarray(b1, dtype=np.float32)
    W2 = np.asarray(W2, dtype=np.float32)
    b2 = np.asarray(b2, dtype=np.float32)
    W3 = np.asarray(W3, dtype=np.float32)
    b3 = np.asarray(b3, dtype=np.float32)

    # ---- host-side routing: group sample indices by expert, pad to CAP.
    idx_per_e = [np.nonzero(head_idx == e)[0] for e in range(E)]
    counts = [len(ix) for ix in idx_per_e]
    assert max(counts) <= CAP, f"expert overflow: {counts}"

    # ---- host-side reorders into DMA-friendly layouts.
    # W1 is quantized to fp8 e3m4 (x W1SCALE so ~N(0, 0.02^2) weights land in
    # e3m4's normal range [0.25, 15.5] instead of its subnormals); scales
    # are folded into the host layer-3 matmul.
    # w1r[e, p, (mh*KD + kd)*128 + h] = e3m4 of W1SCALE*W1[e, kd*128+p, mh*128+h]
    f8e3 = ml_dtypes.float8_e3m4
    w1r = W1.reshape(E, KD, 128, KH, 128).transpose(0, 2, 3, 1, 4)  # [e,p,mh,kd,h]
    w1r = (np.ascontiguousarray(w1r) * W1SCALE).astype(f8e3)
    w1r = w1r.reshape(E, 128, KH * KD * 128)
    # Per-expert importance permutation: sort W2 columns by ascending
    # |W3[col]| row-norm, quantize the first MH2Q*128 to e3m4 (scaled like
    # W1), keep the top 128 in bf16 (also scaled; x64 is exact in bf16).
    # Layer 3 on the host uses the permuted W3, so no inverse is needed.
    NQ = MH2Q * 128
    perms = [np.argsort(np.linalg.norm(W3[e], axis=1), kind="stable") for e in range(E)]
    # h2 row order: the core's LAST expert computes [f8 0:512, bf16
    # 640:768, f8 512:640] (bf16 second-to-last, small f8 tile last); the
    # other expert keeps the natural [f8 0:640, bf16 640:768] order.
    h2order = np.concatenate([np.arange(NQ - 128), np.arange(NQ, H), np.arange(NQ - 128, NQ)])
    w3p = np.stack(
        [
            W3[e][perms[e]][h2order] if e % EPC == EPC - 1 else W3[e][perms[e]]
            for e in range(E)
        ]
    )  # [E, H, T]
    # w2r[e, p, (mh*KH + kh)*128 + h] = W1SCALE * W2[e, kh*128+p, perm[mh*128+h]]
    w2p = np.stack([W2[e][:, perms[e]] for e in range(E)]) * W1SCALE
    w2r = w2p.reshape(E, KH, 128, H).transpose(0, 2, 3, 1)  # [e, p, hcol, kh]
    w2r8 = np.ascontiguousarray(w2r[:, :, :NQ]).astype(f8e3)
    w2r8 = w2r8.reshape(E, 128, MH2Q, 128, KH).transpose(0, 1, 2, 4, 3)
    w2r8 = np.ascontiguousarray(w2r8).reshape(E, 128, MH2Q * KH * 128)
    w2r16 = np.ascontiguousarray(w2r[:, :, NQ:]).astype(bf16)
    w2r16 = w2r16.reshape(E, 128, KH - MH2Q, 128, KH).transpose(0, 1, 2, 4, 3)
    w2r16 = np.ascontiguousarray(w2r16).reshape(E, 128, (KH - MH2Q) * KH * 128)
    # in-kernel bias application was dropped: this problem's b1/b2 are zeros
    # by construction (setup_inputs uses jnp.zeros); guard that assumption.
    assert not b1.any() and not b2.any(), "nonzero b1/b2 not supported"

    in_maps = []
    for c in range(NCORES):
        ge0 = c * EPC
        xgc = np.zeros((128, EPC, KD, CAP), bf16)
        for j in range(EPC):
            ix = idx_per_e[ge0 + j]
            if len(ix):
                # x[ix] : [n, D] -> xT tiles [128, KD, n]
                xt = x[ix].T.reshape(KD, 128, len(ix)).transpose(1, 0, 2)
                xgc[:, j, :, : len(ix)] = xt.astype(bf16)
        in_maps.append(
            {
                "xg": xgc,
                "w1g": w1r[ge0 : ge0 + EPC],
                "w2g8": w2r8[ge0 : ge0 + EPC],
                "w2g16": w2r16[ge0 : ge0 + EPC],
            }
        )

    nc = _get_program()
    res = run_bass_kernel_spmd(nc, in_maps, core_ids=list(range(NCORES)))

    # ---- unshard + host layer 3: out = relu(l2)ᵀ @ W3 + b3, in fp32.
    out = np.empty((B, T), np.float32)
    for c in range(NCORES):
        hgc = res.results[c]["hg"]  # [128, EPC, KH, CAP] bf16
        for j in range(EPC):
            ge = c * EPC + j
            ix = idx_per_e[ge]
            if len(ix):
                # [128, KH, n] -> feature-major [KH*128, n]
                # h2 rows are in permuted column order and carry W1SCALE^2
                # (both layer scales); fold both into the permuted W3.
                h2 = hgc[:, j, :, : len(ix)].astype(np.float32)
                h2 = h2.transpose(1, 0, 2).reshape(H, len(ix))
                out[ix] = h2.T @ (w3p[ge] * (1.0 / (W1SCALE * W1SCALE))) + b3[ge]
    return out



# revision 51
# speedup vs baseline: 1.0298x; 1.0298x over previous
"""MoE routed-classification kernel for Trainium2 (8 NeuronCores, SPMD).

Problem: nn_DINOMIMICClassification — E=16 experts, each a 3-layer MLP
(D=1536 -> H=768 -> H=768 -> T=2, relu after layers 1/2); every sample of
the B=512 batch goes through the expert selected by head_idx[b].

Strategy (expert-parallel, host routing, mixed fp8/bf16 weights):
  - Each of the 8 cores owns 2 experts and receives only the samples routed
    to them (host groups samples by expert, pads each group to CAP=48
    columns; per-expert counts for the fixed input seed max out at 47).
  - The kernel is HBM-stream-bound, so weight BYTES are the roofline:
      * W1 ships as fp8 e3m4 (4 mantissa bits, x64 pre-scale to clear the
        format's subnormal range). The PE accepts fp8-stationary x
        bf16-moving matmuls, so x/h stay bf16.
      * W2 is importance-aware mixed precision: with only T=2 outputs, a
        W2 column's quantization error reaches the output weighted by
        |W3[col]|. The host permutes W2 columns by ascending |W3| row-norm
        (free: layer 3 runs on the host with the permuted W3); the 640
        least-important columns ship as e3m4, the top 128 as bf16.
    Total 4.0 MB/core (vs 7.4 all-bf16). Measured end-to-end rel err
    1.73e-2 against the 2e-2 gate (all-bf16 scores 4.0e-3, all-e3m4 1.95e-2
    without the importance split, 2.07e-2 with an unsorted split).
  - ALL weight chunks ride ONE queue (sync/SP HWDGE), emitted in exactly
    the PE's consumption order; the PE runs one chunk behind the stream.
    Chunk rows are per-partition contiguous (0.8-4.6 KB) for DMA line rate.
  - The PE is ldweights-bound at roughly the stream rate, so W1 is split in
    2 chunks/expert (early PE start) and layer 2 runs expert-serial with
    the bf16 tile second-to-last: after the final weight byte only 6 fp8
    matmuls + a [128,1,48] relu + a 12KB output DMA remain.
  - Layer 3 (768 -> 2) runs on the HOST: the kernel DMAs relu(layer2) back
    (147 KB bf16 total) and the host does the [n,768]@[768,2] einsum in
    fp32. Expert 0's output ships mid-stream on the ACT queue; expert 1's
    bulk ships on ACT while its tail computes, and the final 12 KB rides
    the by-then-idle SP queue so the two issue latencies overlap.
  - Each layer-2 chunk accumulates into its OWN 1-bank PSUM tile (a shared
    tile would serialize matmul->relu->matmul via tile-granular WAR); the
    epilogue is one DVE tensor_scalar max-with-0 (relu + f32->bf16 cast).
  - b1/b2 are zeros for this problem's inputs (asserted); b3 is added on
    the host.
"""

import os

import numpy as np

# Model dims (hardcoded; the grading harness calls kernel() standalone).
E, B, D, H, T = 16, 512, 1536, 768, 2
NCORES = 8
EPC = E // NCORES  # experts per core = 2
CAP = 48  # per-expert routed-sample capacity (actual max is 47)
W1SCALE = 64.0  # pre-scale so fp8(e3m4) W1 uses the format's normal range
KD = D // 128  # 12 contraction tiles for layer 1
KH = H // 128  # 6 contraction tiles for layers 2/3
# W2 mixed precision, importance-aware: the head has only T=2 outputs, so a
# W2 column's quantization error reaches the output weighted by |W3[col]|.
# Host permutes columns by ascending |W3| row-norm; the 5 least-important
# mh-tiles (640 cols) ship as fp8 e3m4, the top tile (128 cols) as bf16.
# Measured end-to-end rel err 1.73e-2 (gate 2e-2); unsorted split would be
# 2.07e-2. The bf16 tile streams LAST per expert, so after the final weight
# byte only 6 matmuls + a [128,1,48] relu + a 12KB DMA remain.
MH2Q = 5  # e3m4 mh-tiles per expert

_CACHE = {}


def _build_program():
    """Build the (single, SPMD) Bass program run on every core."""
    from contextlib import ExitStack

    import concourse.mybir as mybir
    import concourse.tile as tile
    from concourse import bacc

    f32 = mybir.dt.float32
    bf16 = mybir.dt.bfloat16
    f8e3 = mybir.dt.float8e3
    # Bacc (not raw Bass): its compile() legalization splits multi-sem waits
    # into EventSemaphore sequencer ops — TPB instructions have a single
    # hardware wait slot and walrus rejects >1 ("Too many sync wait commands").
    nc = bacc.Bacc("TRN2")

    # xg[p, e, kd, c]: bf16 routed samples, transposed per expert
    xg = nc.dram_tensor("xg", [128, EPC, KD, CAP], bf16, kind="ExternalInput")
    # w1g[e, p, (mh, kd*128+h)] = e3m4 of W1SCALE*W1[ge, kd*128+p, mh*128+h].
    # fp8 e3m4 (4 mantissa bits) halves W1's HBM traffic vs bf16; the PE
    # accepts mixed fp8-weights x bf16-moving matmuls. Two chunks per
    # expert: the PE is ldweights-bound at roughly the DMA stream rate, so
    # it must start on e0's first half as early as possible or it finishes
    # ~1.3us after the last weight byte.
    w1g = nc.dram_tensor("w1g", [EPC, 128, KH * KD * 128], f8e3, kind="ExternalInput")
    # w2g8/w2g16[e, p, (mh, kh*128+h)] = W2SCALE*W2[ge, kh*128+p, perm[mh*128+h]]:
    # flat per-expert rows; mh counts permuted-column tiles (0..4 fp8, 5 bf16).
    w2g8 = nc.dram_tensor("w2g8", [EPC, 128, MH2Q * KH * 128], f8e3, kind="ExternalInput")
    w2g16 = nc.dram_tensor("w2g16", [EPC, 128, (KH - MH2Q) * KH * 128], bf16, kind="ExternalInput")
    # hg[p, e, kh, c] = relu(layer2) activations, feature kh*128+p
    hg = nc.dram_tensor("hg", [128, EPC, KH, CAP], bf16, kind="ExternalOutput")

    with tile.TileContext(nc) as tc, ExitStack() as ctx:
        const_pool = ctx.enter_context(tc.tile_pool(name="const", bufs=1))
        # bufs >= number of live tiles per tag: any reuse would add a WAR
        # wait that stalls the in-order weight queue mid-stream.
        w1_pool = ctx.enter_context(tc.tile_pool(name="w1", bufs=3))
        w2_pool = ctx.enter_context(tc.tile_pool(name="w2", bufs=5))
        h_pool = ctx.enter_context(tc.tile_pool(name="h", bufs=EPC))

        # 2 layer-1 tiles + 5 layer-2 chunk tiles: every PSUM allocation
        # lives in its own bank (7 of 8); no buffer reuse -> no WAR stalls.
        psL_pool = ctx.enter_context(tc.tile_pool(name="psL", bufs=7, space="PSUM"))

        # x ships per expert: e0's slice leads the SP queue (the PE's first
        # matmul waits on it); e1's rides the ACT queue early, where the
        # PE is busy anyway. Total DMA time is queue-order-invariant.
        xsb = const_pool.tile([128, EPC, KD, CAP], bf16)
        nc.sync.dma_start(out=xsb[:, 0], in_=xg[:, 0])
        nc.scalar.dma_start(out=xsb[:, 1], in_=xg[:, 1])

        # Weight stream: ONE queue (sync), emitted in consumption order.
        # Layer-2 chunks run expert-serial (all of e0, then e1) so e0's
        # output DMA overlaps e1's weight stream and only e1's small bf16
        # tail remains after the last byte. Every chunk has its own buffer;
        # all DMAs are issued up front (no reuse waits).
        # W1 chunking is asymmetric: e0 gets a small 1-mh-tile head chunk so
        # the ldweights-bound PE starts as early as possible; everything
        # after rides maximal-size chunks (bigger descriptors = higher DMA
        # rate; mid-stream the PE runs behind the DMA anyway).
        # e0's 1-mh head chunk rides the ACT queue so its DMA issue and
        # completion receipt overlap x-e0's on SP - the PE's first
        # ldweights fires ~0.5us earlier.
        W1CH = ((1, 5), (6,))
        w1sb = []
        for e in range(EPC):
            chunks = []
            off = 0
            for mhc in W1CH[e]:
                t = w1_pool.tile([128, mhc, KD * 128], f8e3, tag="w1", name=f"w1_{e}_{off}")
                lo = off * KD * 128
                q = nc.scalar if (e == 0 and off == 0) else nc.sync
                q.dma_start(out=t, in_=w1g[e][:, lo : lo + mhc * KD * 128])
                chunks.append((t, off, mhc))
                off += mhc
            w1sb.append(chunks)
        # Per-expert W2 chunk order: [f8 mh0-3][bf16 mh5][f8 mh4]. The LAST
        # chunk is fp8 (ldweights at 2 cols/cycle, ~half the bf16 cost), so
        # after the final byte only 6 cheap matmuls + a small relu remain.
        # h2 rows are assigned in COMPUTE order (f8 mh0-3 -> rows 0-3, bf16
        # -> row 4, f8 mh4 -> row 5), so the "everything but the last
        # chunk" output slice hg[:, e, :5, :] is contiguous. The host's
        # permuted-W3 row order matches this mapping.
        w2sb = []  # [e] -> list of (tile, h2-row-offset, mh-count)
        for e in range(EPC):
            chunks = []
            if e < EPC - 1:
                # not the global tail: one maximal fp8 chunk + the bf16 tile
                t8 = w2_pool.tile([128, MH2Q, KH * 128], f8e3, tag="w2", name=f"w2q_{e}")
                nc.sync.dma_start(out=t8, in_=w2g8[e])
                chunks.append((t8, 0, MH2Q))
                t16 = w2_pool.tile([128, KH - MH2Q, KH * 128], bf16, tag="w2", name=f"w2b_{e}")
                nc.sync.dma_start(out=t16, in_=w2g16[e])
                chunks.append((t16, MH2Q, KH - MH2Q))
            else:
                # global tail expert: bf16 second-to-last, small fp8 last
                t8a = w2_pool.tile([128, MH2Q - 1, KH * 128], f8e3, tag="w2", name=f"w2qa_{e}")
                nc.sync.dma_start(out=t8a, in_=w2g8[e][:, : (MH2Q - 1) * KH * 128])
                chunks.append((t8a, 0, MH2Q - 1))
                t16 = w2_pool.tile([128, KH - MH2Q, KH * 128], bf16, tag="w2", name=f"w2b_{e}")
                nc.sync.dma_start(out=t16, in_=w2g16[e])
                chunks.append((t16, MH2Q - 1, KH - MH2Q))
                t8b = w2_pool.tile([128, 1, KH * 128], f8e3, tag="w2", name=f"w2qb_{e}")
                nc.sync.dma_start(out=t8b, in_=w2g8[e][:, (MH2Q - 1) * KH * 128 :])
                chunks.append((t8b, KH - 1, 1))
            w2sb.append(chunks)

        # ---- layer 1 (both experts), relu epilogue. Consumption matches
        # the DMA emission order so the PE runs one chunk behind the stream.
        h1 = [h_pool.tile([128, KH, CAP], bf16, tag="h", name=f"h1_{e}") for e in range(EPC)]
        PS1 = [psL_pool.tile([128, KH, CAP], f32, tag="psL", name=f"ps1_{e}") for e in range(EPC)]
        for e in range(EPC):
            for w, off, mhc in w1sb[e]:
                for j in range(mhc):
                    for k in range(KD):
                        nc.tensor.matmul(
                            PS1[e][:, off + j, :],
                            w[:, j, k * 128 : (k + 1) * 128],
                            xsb[:, e, k, :],
                            start=(k == 0),
                            stop=(k == KD - 1),
                        )
            # relu with implicit f32->bf16 cast
            nc.vector.tensor_scalar_max(h1[e], PS1[e], 0.0)

        # ---- layer 2, expert-serial, per-chunk relu epilogue + output DMA.
        # Each chunk gets its OWN psum tile: a shared tile would give the
        # next chunk's matmuls a tile-granular WAR wait on this chunk's
        # relu, serializing matmul->relu->matmul at the stream tail.
        h2 = const_pool.tile([128, EPC, KH, CAP], bf16, tag="h2")
        for e in range(EPC):
            for ci, (w, off, mhc) in enumerate(w2sb[e]):
                ps = psL_pool.tile([128, mhc, CAP], f32, tag="psL", name=f"ps2_{e}_{off}")
                for j in range(mhc):
                    for k in range(KH):
                        nc.tensor.matmul(
                            ps[:, j, :],
                            w[:, j, k * 128 : (k + 1) * 128],
                            h1[e][:, k, :],
                            start=(k == 0),
                            stop=(k == KH - 1),
                        )
                # (Running the last chunk's relu on GpSimd to dodge the DVE
                # queue was tried: walrus rejects it - no GpSimd PSUM path.)
                nc.vector.tensor_scalar_max(h2[:, e, off : off + mhc, :], ps, 0.0)
            if e < EPC - 1:
                # whole expert ships right after its last relu, on the ACT
                # queue (the SP queue is still carrying e1's weights)
                nc.scalar.dma_start(out=hg[:, e, :, :], in_=h2[:, e, :, :])
            else:
                # last expert: bulk (5 mh-tiles) ships on ACT while the bf16
                # tail chunk computes; the final 12KB rides the (by now
                # idle) SP queue so the two DMAs' issue latencies overlap.
                # (Emitting these OUTSIDE the TileContext to overlap the
                # end-of-NEFF teardown was tried and crashes walrus codegen.)
                nc.scalar.dma_start(out=hg[:, e, :MH2Q, :], in_=h2[:, e, :MH2Q, :])
                nc.sync.dma_start(out=hg[:, e, MH2Q:, :], in_=h2[:, e, MH2Q:, :])

    nc.finalize()
    return nc


def _get_program():
    if "nc" not in _CACHE:
        _CACHE["nc"] = _build_program()
    return _CACHE["nc"]


def kernel(x, head_idx, W1, b1, W2, b2, W3, b3):
    # Make sure the axon jax platform is reachable (the Bass program executes
    # via PJRT on the 8 tunneled NeuronCores).
    if os.environ.get("JAX_PLATFORMS") not in (None, ""):
        if "axon" not in os.environ["JAX_PLATFORMS"]:
            os.environ["JAX_PLATFORMS"] = ""

    import ml_dtypes

    from concourse.bass_utils import run_bass_kernel_spmd

    bf16 = ml_dtypes.bfloat16
    x = np.ascontiguousarray(np.asarray(x, dtype=np.float32))
    head_idx = np.asarray(head_idx, dtype=np.int32)
    W1 = np.asarray(W1, dtype=np.float32)
    b1 = np.asarray(b1, dtype=np.float32)
    W2 = np.asarray(W2, dtype=np.float32)
    b2 = np.asarray(b2, dtype=np.float32)
    W3 = np.asarray(W3, dtype=np.float32)
    b3 = np.asarray(b3, dtype=np.float32)

    # ---- host-side routing: group sample indices by expert, pad to CAP.
    idx_per_e = [np.nonzero(head_idx == e)[0] for e in range(E)]
    counts = [len(ix) for ix in idx_per_e]
    assert max(counts) <= CAP, f"expert overflow: {counts}"

    # ---- host-side reorders into DMA-friendly layouts.
    # W1 is quantized to fp8 e3m4 (x W1SCALE so ~N(0, 0.02^2) weights land in
    # e3m4's normal range [0.25, 15.5] instead of its subnormals); scales
    # are folded into the host layer-3 matmul.
    # w1r[e, p, (mh*KD + kd)*128 + h] = e3m4 of W1SCALE*W1[e, kd*128+p, mh*128+h]
    f8e3 = ml_dtypes.float8_e3m4
    w1r = W1.reshape(E, KD, 128, KH, 128).transpose(0, 2, 3, 1, 4)  # [e,p,mh,kd,h]
    w1r = (np.ascontiguousarray(w1r) * W1SCALE).astype(f8e3)
    w1r = w1r.reshape(E, 128, KH * KD * 128)
    # Per-expert importance permutation: sort W2 columns by ascending
    # |W3[col]| row-norm, quantize the first MH2Q*128 to e3m4 (scaled like
    # W1), keep the top 128 in bf16 (also scaled; x64 is exact in bf16).
    # Layer 3 on the host uses the permuted W3, so no inverse is needed.
    NQ = MH2Q * 128
    perms = [np.argsort(np.linalg.norm(W3[e], axis=1), kind="stable") for e in range(E)]
    # h2 row order: the core's LAST expert computes [f8 0:512, bf16
    # 640:768, f8 512:640] (bf16 second-to-last, small f8 tile last); the
    # other expert keeps the natural [f8 0:640, bf16 640:768] order.
    h2order = np.concatenate([np.arange(NQ - 128), np.arange(NQ, H), np.arange(NQ - 128, NQ)])
    w3p = np.stack(
        [
            W3[e][perms[e]][h2order] if e % EPC == EPC - 1 else W3[e][perms[e]]
            for e in range(E)
        ]
    )  # [E, H, T]
    # w2r[e, p, (mh*KH + kh)*128 + h] = W1SCALE * W2[e, kh*128+p, perm[mh*128+h]]
    w2p = np.stack([W2[e][:, perms[e]] for e in range(E)]) * W1SCALE
    w2r = w2p.reshape(E, KH, 128, H).transpose(0, 2, 3, 1)  # [e, p, hcol, kh]
    w2r8 = np.ascontiguousarray(w2r[:, :, :NQ]).astype(f8e3)
    w2r8 = w2r8.reshape(E, 128, MH2Q, 128, KH).transpose(0, 1, 2, 4, 3)
    w2r8 = np.ascontiguousarray(w2r8).reshape(E, 128, MH2Q * KH * 128)
    w2r16 = np.ascontiguousarray(w2r[:, :, NQ:]).astype(bf16)
    w2r16 = w2r16.reshape(E, 128, KH - MH2Q, 128, KH).transpose(0, 1, 2, 4, 3)
    w2r16 = np.ascontiguousarray(w2r16).reshape(E, 128, (KH - MH2Q) * KH * 128)
    # in-kernel bias application was dropped: this problem's b1/b2 are zeros
    # by construction (setup_inputs uses jnp.zeros); guard that assumption.
    assert not b1.any() and not b2.any(), "nonzero b1/b2 not supported"

    in_maps = []
    for c in range(NCORES):
        ge0 = c * EPC
        xgc = np.zeros((128, EPC, KD, CAP), bf16)
        for j in range(EPC):
            ix = idx_per_e[ge0 + j]
            if len(ix):
                # x[ix] : [n, D] -> xT tiles [128, KD, n]
                xt = x[ix].T.reshape(KD, 128, len(ix)).transpose(1, 0, 2)
                xgc[:, j, :, : len(ix)] = xt.astype(bf16)
        in_maps.append(
            {
                "xg": xgc,
                "w1g": w1r[ge0 : ge0 + EPC],
                "w2g8": w2r8[ge0 : ge0 + EPC],
                "w2g16": w2r16[ge0 : ge0 + EPC],
            }
        )

    nc = _get_program()
    res = run_bass_kernel_spmd(nc, in_maps, core_ids=list(range(NCORES)))

    # ---- unshard + host layer 3: out = relu(l2)ᵀ @ W3 + b3, in fp32.
    out = np.empty((B, T), np.float32)
    for c in range(NCORES):
        hgc = res.results[c]["hg"]  # [128, EPC, KH, CAP] bf16
        for j in range(EPC):
            ge = c * EPC + j
            ix = idx_per_e[ge]
            if len(ix):
                # [128, KH, n] -> feature-major [KH*128, n]
                # h2 rows are in permuted column order and carry W1SCALE^2
                # (both layer scales); fold both into the permuted W3.
                h2 = hgc[:, j, :, : len(ix)].astype(np.float32)
                h2 = h2.transpose(1, 0, 2).reshape(H, len(ix))
                out[ix] = h2.T @ (w3p[ge] * (1.0 / (W1SCALE * W1SCALE))) + b3[ge]
    return out



# revision 53
# speedup vs baseline: 1.0880x; 1.0565x over previous
"""MoE routed-classification kernel for Trainium2 (8 NeuronCores, SPMD).

Problem: nn_DINOMIMICClassification — E=16 experts, each a 3-layer MLP
(D=1536 -> H=768 -> H=768 -> T=2, relu after layers 1/2); every sample of
the B=512 batch goes through the expert selected by head_idx[b].

Strategy (expert-parallel, host routing, mixed fp8/bf16 weights):
  - Each of the 8 cores owns 2 experts and receives only the samples routed
    to them (host groups samples by expert, pads each group to CAP=48
    columns; per-expert counts for the fixed input seed max out at 47).
  - The kernel is HBM-stream-bound, so weight BYTES are the roofline:
      * W1 ships as fp8 e3m4 (4 mantissa bits, x64 pre-scale to clear the
        format's subnormal range). The PE accepts fp8-stationary x
        bf16-moving matmuls, so x/h stay bf16.
      * W2 is importance-aware mixed precision: with only T=2 outputs, a
        W2 column's quantization error reaches the output weighted by
        |W3[col]|. The host permutes W2 columns by ascending |W3| row-norm
        (free: layer 3 runs on the host with the permuted W3); the 640
        least-important columns ship as e3m4, the top 128 as bf16.
    Total 4.0 MB/core (vs 7.4 all-bf16). Measured end-to-end rel err
    1.73e-2 against the 2e-2 gate (all-bf16 scores 4.0e-3, all-e3m4 1.95e-2
    without the importance split, 2.07e-2 with an unsorted split).
  - ALL weight chunks ride ONE queue (sync/SP HWDGE), emitted in exactly
    the PE's consumption order; the PE runs one chunk behind the stream.
    Chunk rows are per-partition contiguous (0.8-4.6 KB) for DMA line rate.
  - The PE is ldweights-bound at roughly the stream rate, so W1 is split in
    2 chunks/expert (early PE start) and layer 2 runs expert-serial with
    the bf16 tile second-to-last: after the final weight byte only 6 fp8
    matmuls + a [128,1,48] relu + a 12KB output DMA remain.
  - Layer 3 (768 -> 2) runs on the HOST: the kernel DMAs relu(layer2) back
    (147 KB bf16 total) and the host does the [n,768]@[768,2] einsum in
    fp32. Expert 0's output ships mid-stream on the ACT queue; expert 1's
    bulk ships on ACT while its tail computes, and the final 12 KB rides
    the by-then-idle SP queue so the two issue latencies overlap.
  - Each layer-2 chunk accumulates into its OWN 1-bank PSUM tile (a shared
    tile would serialize matmul->relu->matmul via tile-granular WAR); the
    epilogue is one DVE tensor_scalar max-with-0 (relu + f32->bf16 cast).
  - b1/b2 are zeros for this problem's inputs (asserted); b3 is added on
    the host.
"""

import os

import numpy as np

# Model dims (hardcoded; the grading harness calls kernel() standalone).
E, B, D, H, T = 16, 512, 1536, 768, 2
NCORES = 8
EPC = E // NCORES  # experts per core = 2
CAP = 48  # per-expert routed-sample capacity (actual max is 47)
W1SCALE = 64.0  # pre-scale so fp8(e3m4) W1 uses the format's normal range
KD = D // 128  # 12 contraction tiles for layer 1
KH = H // 128  # 6 contraction tiles for layers 2/3
# W2 mixed precision, importance-aware: the head has only T=2 outputs, so a
# W2 column's quantization error reaches the output weighted by |W3[col]|.
# Host permutes columns by ascending |W3| row-norm; the 5 least-important
# mh-tiles (640 cols) ship as fp8 e3m4, the top tile (128 cols) as bf16.
# Measured end-to-end rel err 1.73e-2 (gate 2e-2); unsorted split would be
# 2.07e-2. The bf16 tile streams LAST per expert, so after the final weight
# byte only 6 matmuls + a [128,1,48] relu + a 12KB DMA remain.
MH2Q = 5  # e3m4 mh-tiles per expert

_CACHE = {}


def _build_program():
    """Build the (single, SPMD) Bass program run on every core."""
    from contextlib import ExitStack

    import concourse.mybir as mybir
    import concourse.tile as tile
    from concourse import bacc

    f32 = mybir.dt.float32
    bf16 = mybir.dt.bfloat16
    f8e3 = mybir.dt.float8e3
    # Bacc (not raw Bass): its compile() legalization splits multi-sem waits
    # into EventSemaphore sequencer ops — TPB instructions have a single
    # hardware wait slot and walrus rejects >1 ("Too many sync wait commands").
    nc = bacc.Bacc("TRN2")

    # xg[p, e, kd, c]: bf16 routed samples, transposed per expert
    xg = nc.dram_tensor("xg", [128, EPC, KD, CAP], bf16, kind="ExternalInput")
    # w1g[e, p, (mh, kd*128+h)] = e3m4 of W1SCALE*W1[ge, kd*128+p, mh*128+h].
    # fp8 e3m4 (4 mantissa bits) halves W1's HBM traffic vs bf16; the PE
    # accepts mixed fp8-weights x bf16-moving matmuls. Two chunks per
    # expert: the PE is ldweights-bound at roughly the DMA stream rate, so
    # it must start on e0's first half as early as possible or it finishes
    # ~1.3us after the last weight byte.
    w1g = nc.dram_tensor("w1g", [EPC, 128, KH * KD * 128], f8e3, kind="ExternalInput")
    # w2g8/w2g16[e, p, (mh, kh*128+h)] = W2SCALE*W2[ge, kh*128+p, perm[mh*128+h]]:
    # flat per-expert rows; mh counts permuted-column tiles (0..4 fp8, 5 bf16).
    w2g8 = nc.dram_tensor("w2g8", [EPC, 128, MH2Q * KH * 128], f8e3, kind="ExternalInput")
    w2g16 = nc.dram_tensor("w2g16", [EPC, 128, (KH - MH2Q) * KH * 128], bf16, kind="ExternalInput")
    # hg[p, e, kh, c] = relu(layer2) activations, feature kh*128+p
    hg = nc.dram_tensor("hg", [128, EPC, KH, CAP], bf16, kind="ExternalOutput")

    with tile.TileContext(nc) as tc, ExitStack() as ctx:
        const_pool = ctx.enter_context(tc.tile_pool(name="const", bufs=1))
        # bufs >= number of live tiles per tag: any reuse would add a WAR
        # wait that stalls the in-order weight queue mid-stream.
        w1_pool = ctx.enter_context(tc.tile_pool(name="w1", bufs=3))
        w2_pool = ctx.enter_context(tc.tile_pool(name="w2", bufs=5))
        h_pool = ctx.enter_context(tc.tile_pool(name="h", bufs=EPC))

        # 2 layer-1 tiles + 5 layer-2 chunk tiles: every PSUM allocation
        # lives in its own bank (7 of 8); no buffer reuse -> no WAR stalls.
        psL_pool = ctx.enter_context(tc.tile_pool(name="psL", bufs=7, space="PSUM"))

        # x ships per expert: e0's slice leads the queue (the PE's first
        # matmul waits on it), e1's rides between the W1 chunks where the
        # PE is busy anyway. Total DMA time is queue-order-invariant.
        # (Putting the head transfers on the ACT queue was tried: the ACT
        # queue's first bytes land ~3us later than SP's, tripling PE lag.)
        xsb = const_pool.tile([128, EPC, KD, CAP], bf16)
        nc.sync.dma_start(out=xsb[:, 0], in_=xg[:, 0])

        # Weight stream: ONE queue (sync), emitted in consumption order.
        # Layer-2 chunks run expert-serial (all of e0, then e1) so e0's
        # output DMA overlaps e1's weight stream and only e1's small bf16
        # tail remains after the last byte. Every chunk has its own buffer;
        # all DMAs are issued up front (no reuse waits).
        # W1 chunking is asymmetric: e0 gets a small 1-mh-tile head chunk so
        # the ldweights-bound PE starts as early as possible; everything
        # after rides maximal-size chunks (bigger descriptors = higher DMA
        # rate; mid-stream the PE runs behind the DMA anyway).
        W1CH = ((1, 5), (6,))
        w1sb = []
        for e in range(EPC):
            chunks = []
            off = 0
            for mhc in W1CH[e]:
                t = w1_pool.tile([128, mhc, KD * 128], f8e3, tag="w1", name=f"w1_{e}_{off}")
                lo = off * KD * 128
                nc.sync.dma_start(out=t, in_=w1g[e][:, lo : lo + mhc * KD * 128])
                chunks.append((t, off, mhc))
                off += mhc
            if e == 0:
                # e1's x slice rides after e0's W1 (PE busy with e0 by then)
                nc.sync.dma_start(out=xsb[:, 1], in_=xg[:, 1])
            w1sb.append(chunks)
        # Per-expert W2 chunk order: [f8 mh0-3][bf16 mh5][f8 mh4]. The LAST
        # chunk is fp8 (ldweights at 2 cols/cycle, ~half the bf16 cost), so
        # after the final byte only 6 cheap matmuls + a small relu remain.
        # h2 rows are assigned in COMPUTE order (f8 mh0-3 -> rows 0-3, bf16
        # -> row 4, f8 mh4 -> row 5), so the "everything but the last
        # chunk" output slice hg[:, e, :5, :] is contiguous. The host's
        # permuted-W3 row order matches this mapping.
        w2sb = []  # [e] -> list of (tile, h2-row-offset, mh-count)
        for e in range(EPC):
            chunks = []
            if e < EPC - 1:
                # not the global tail: one maximal fp8 chunk + the bf16 tile
                t8 = w2_pool.tile([128, MH2Q, KH * 128], f8e3, tag="w2", name=f"w2q_{e}")
                nc.sync.dma_start(out=t8, in_=w2g8[e])
                chunks.append((t8, 0, MH2Q))
                t16 = w2_pool.tile([128, KH - MH2Q, KH * 128], bf16, tag="w2", name=f"w2b_{e}")
                nc.sync.dma_start(out=t16, in_=w2g16[e])
                chunks.append((t16, MH2Q, KH - MH2Q))
            else:
                # global tail expert: bf16 second-to-last, small fp8 last
                t8a = w2_pool.tile([128, MH2Q - 1, KH * 128], f8e3, tag="w2", name=f"w2qa_{e}")
                nc.sync.dma_start(out=t8a, in_=w2g8[e][:, : (MH2Q - 1) * KH * 128])
                chunks.append((t8a, 0, MH2Q - 1))
                t16 = w2_pool.tile([128, KH - MH2Q, KH * 128], bf16, tag="w2", name=f"w2b_{e}")
                nc.sync.dma_start(out=t16, in_=w2g16[e])
                chunks.append((t16, MH2Q - 1, KH - MH2Q))
                t8b = w2_pool.tile([128, 1, KH * 128], f8e3, tag="w2", name=f"w2qb_{e}")
                nc.sync.dma_start(out=t8b, in_=w2g8[e][:, (MH2Q - 1) * KH * 128 :])
                chunks.append((t8b, KH - 1, 1))
            w2sb.append(chunks)

        # ---- layer 1 (both experts), relu epilogue. Consumption matches
        # the DMA emission order so the PE runs one chunk behind the stream.
        h1 = [h_pool.tile([128, KH, CAP], bf16, tag="h", name=f"h1_{e}") for e in range(EPC)]
        PS1 = [psL_pool.tile([128, KH, CAP], f32, tag="psL", name=f"ps1_{e}") for e in range(EPC)]
        for e in range(EPC):
            for w, off, mhc in w1sb[e]:
                for j in range(mhc):
                    for k in range(KD):
                        nc.tensor.matmul(
                            PS1[e][:, off + j, :],
                            w[:, j, k * 128 : (k + 1) * 128],
                            xsb[:, e, k, :],
                            start=(k == 0),
                            stop=(k == KD - 1),
                        )
            # relu with implicit f32->bf16 cast
            nc.vector.tensor_scalar_max(h1[e], PS1[e], 0.0)

        # ---- layer 2, expert-serial, per-chunk relu epilogue + output DMA.
        # Each chunk gets its OWN psum tile: a shared tile would give the
        # next chunk's matmuls a tile-granular WAR wait on this chunk's
        # relu, serializing matmul->relu->matmul at the stream tail.
        h2 = const_pool.tile([128, EPC, KH, CAP], bf16, tag="h2")
        for e in range(EPC):
            for ci, (w, off, mhc) in enumerate(w2sb[e]):
                ps = psL_pool.tile([128, mhc, CAP], f32, tag="psL", name=f"ps2_{e}_{off}")
                for j in range(mhc):
                    for k in range(KH):
                        nc.tensor.matmul(
                            ps[:, j, :],
                            w[:, j, k * 128 : (k + 1) * 128],
                            h1[e][:, k, :],
                            start=(k == 0),
                            stop=(k == KH - 1),
                        )
                # (Running the last chunk's relu on GpSimd to dodge the DVE
                # queue was tried: walrus rejects it - no GpSimd PSUM path.)
                nc.vector.tensor_scalar_max(h2[:, e, off : off + mhc, :], ps, 0.0)
            if e < EPC - 1:
                # whole expert ships right after its last relu, on the ACT
                # queue (the SP queue is still carrying e1's weights)
                nc.scalar.dma_start(out=hg[:, e, :, :], in_=h2[:, e, :, :])
            else:
                # last expert: bulk (5 mh-tiles) ships on ACT while the bf16
                # tail chunk computes; the final 12KB rides the (by now
                # idle) SP queue so the two DMAs' issue latencies overlap.
                # (Emitting these OUTSIDE the TileContext to overlap the
                # end-of-NEFF teardown was tried and crashes walrus codegen.)
                nc.scalar.dma_start(out=hg[:, e, :MH2Q, :], in_=h2[:, e, :MH2Q, :])
                nc.sync.dma_start(out=hg[:, e, MH2Q:, :], in_=h2[:, e, MH2Q:, :])

    nc.finalize()
    return nc


def _get_program():
    if "nc" not in _CACHE:
        _CACHE["nc"] = _build_program()
    return _CACHE["nc"]


def kernel(x, head_idx, W1, b1, W2, b2, W3, b3):
    # Make sure the axon jax platform is reachable (the Bass program executes
    # via PJRT on the 8 tunneled NeuronCores).
    if os.environ.get("JAX_PLATFORMS") not in (None, ""):
        if "axon" not in os.environ["JAX_PLATFORMS"]:
            os.environ["JAX_PLATFORMS"] = ""

    import ml_dtypes

    from concourse.bass_utils import run_bass_kernel_spmd

    bf16 = ml_dtypes.bfloat16
    x = np.ascontiguousarray(np.asarray(x, dtype=np.float32))
    head_idx = np.asarray(head_idx, dtype=np.int32)
    W1 = np.asarray(W1, dtype=np.float32)
    b1 = np.asarray(b1, dtype=np.float32)
    W2 = np.asarray(W2, dtype=np.float32)
    b2 = np.asarray(b2, dtype=np.float32)
    W3 = np.asarray(W3, dtype=np.float32)
    b3 = np.asarray(b3, dtype=np.float32)

    # ---- host-side routing: group sample indices by expert, pad to CAP.
    idx_per_e = [np.nonzero(head_idx == e)[0] for e in range(E)]
    counts = [len(ix) for ix in idx_per_e]
    assert max(counts) <= CAP, f"expert overflow: {counts}"

    # ---- host-side reorders into DMA-friendly layouts.
    # W1 is quantized to fp8 e3m4 (x W1SCALE so ~N(0, 0.02^2) weights land in
    # e3m4's normal range [0.25, 15.5] instead of its subnormals); scales
    # are folded into the host layer-3 matmul.
    # w1r[e, p, (mh*KD + kd)*128 + h] = e3m4 of W1SCALE*W1[e, kd*128+p, mh*128+h]
    f8e3 = ml_dtypes.float8_e3m4
    w1r = W1.reshape(E, KD, 128, KH, 128).transpose(0, 2, 3, 1, 4)  # [e,p,mh,kd,h]
    w1r = (np.ascontiguousarray(w1r) * W1SCALE).astype(f8e3)
    w1r = w1r.reshape(E, 128, KH * KD * 128)
    # Per-expert importance permutation: sort W2 columns by ascending
    # |W3[col]| row-norm, quantize the first MH2Q*128 to e3m4 (scaled like
    # W1), keep the top 128 in bf16 (also scaled; x64 is exact in bf16).
    # Layer 3 on the host uses the permuted W3, so no inverse is needed.
    NQ = MH2Q * 128
    perms = [np.argsort(np.linalg.norm(W3[e], axis=1), kind="stable") for e in range(E)]
    # h2 row order: the core's LAST expert computes [f8 0:512, bf16
    # 640:768, f8 512:640] (bf16 second-to-last, small f8 tile last); the
    # other expert keeps the natural [f8 0:640, bf16 640:768] order.
    h2order = np.concatenate([np.arange(NQ - 128), np.arange(NQ, H), np.arange(NQ - 128, NQ)])
    w3p = np.stack(
        [
            W3[e][perms[e]][h2order] if e % EPC == EPC - 1 else W3[e][perms[e]]
            for e in range(E)
        ]
    )  # [E, H, T]
    # w2r[e, p, (mh*KH + kh)*128 + h] = W1SCALE * W2[e, kh*128+p, perm[mh*128+h]]
    w2p = np.stack([W2[e][:, perms[e]] for e in range(E)]) * W1SCALE
    w2r = w2p.reshape(E, KH, 128, H).transpose(0, 2, 3, 1)  # [e, p, hcol, kh]
    w2r8 = np.ascontiguousarray(w2r[:, :, :NQ]).astype(f8e3)
    w2r8 = w2r8.reshape(E, 128, MH2Q, 128, KH).transpose(0, 1, 2, 4, 3)
    w2r8 = np.ascontiguousarray(w2r8).reshape(E, 128, MH2Q * KH * 128)
    w2r16 = np.ascontiguousarray(w2r[:, :, NQ:]).astype(bf16)
    w2r16 = w2r16.reshape(E, 128, KH - MH2Q, 128, KH).transpose(0, 1, 2, 4, 3)
    w2r16 = np.ascontiguousarray(w2r16).reshape(E, 128, (KH - MH2Q) * KH * 128)
    # in-kernel bias application was dropped: this problem's b1/b2 are zeros
    # by construction (setup_inputs uses jnp.zeros); guard that assumption.
    assert not b1.any() and not b2.any(), "nonzero b1/b2 not supported"

    in_maps = []
    for c in range(NCORES):
        ge0 = c * EPC
        xgc = np.zeros((128, EPC, KD, CAP), bf16)
        for j in range(EPC):
            ix = idx_per_e[ge0 + j]
            if len(ix):
                # x[ix] : [n, D] -> xT tiles [128, KD, n]
                xt = x[ix].T.reshape(KD, 128, len(ix)).transpose(1, 0, 2)
                xgc[:, j, :, : len(ix)] = xt.astype(bf16)
        in_maps.append(
            {
                "xg": xgc,
                "w1g": w1r[ge0 : ge0 + EPC],
                "w2g8": w2r8[ge0 : ge0 + EPC],
                "w2g16": w2r16[ge0 : ge0 + EPC],
            }
        )

    nc = _get_program()
    res = run_bass_kernel_spmd(nc, in_maps, core_ids=list(range(NCORES)))

    # ---- unshard + host layer 3: out = relu(l2)ᵀ @ W3 + b3, in fp32.
    out = np.empty((B, T), np.float32)
    for c in range(NCORES):
        hgc = res.results[c]["hg"]  # [128, EPC, KH, CAP] bf16
        for j in range(EPC):
            ge = c * EPC + j
            ix = idx_per_e[ge]
            if len(ix):
                # [128, KH, n] -> feature-major [KH*128, n]
                # h2 rows are in permuted column order and carry W1SCALE^2
                # (both layer scales); fold both into the permuted W3.
                h2 = hgc[:, j, :, : len(ix)].astype(np.float32)
                h2 = h2.transpose(1, 0, 2).reshape(H, len(ix))
                out[ix] = h2.T @ (w3p[ge] * (1.0 / (W1SCALE * W1SCALE))) + b3[ge]
    return out

